# revision 23
# baseline (speedup 1.0000x reference)
"""2-layer GATv2 + global mean pool + linear head, on 8 Trainium2 NeuronCores.

Strategy (v2: dst-aligned "coloring" schedule, identity scatter):
  - Nodes sorted by in-degree desc, dealt into blocks of 1024 = 8 cores x 128
    partitions; block g, deal-slot p, core c.  Grid column s of group g holds,
    at partition p, the s-th in-edge of node (c,g,p).  The scatter indicator is
    the IDENTITY for every chunk -> PE weights loaded once; no xr gather
    (xr[dst] broadcasts per group); no is_equal.
  - Tables XL = x@W_l + (b_l + bias), XR = x@W_r + (b_r - bias), extended with
    8 columns carrying 0.2*(z@att_blk) linear score part:
    leakyrelu decomposed as 0.2*z + 0.8*relu(z); relu runs on ACT.
  - Pad slots gather a -1e4 table row -> exp underflows to exact 0 (no masks).
  - out_pre_elu = (sum_s p_s*z_s) * rec - XR[dst]   (softmax sums to 1).
  - Per-group batched DVE ops in (c,h) feature layout; tree reduction for the
    att-weighted score sum.
"""

import sys

for _p in ("/opt/trn_rl_repo",):
    if _p not in sys.path:
        sys.path.insert(0, _p)

import numpy as np
import ml_dtypes

BF = ml_dtypes.bfloat16

import concourse.bass as bass
import concourse.mybir as mybir
from concourse.tile import TileContext
from concourse.bass_utils import run_bass_kernel_spmd
from concourse.masks import make_identity

F32 = mybir.dt.float32
BF16 = mybir.dt.bfloat16
I32 = mybir.dt.int32
P = 128
NCORES = 8
NUM_GRAPHS = 64
D = 128
EXT = 8
ROWW = D + EXT     # 136: 128 feats + 8 linear-score cols
VCAP = 24          # max grid columns per vgroup (SBUF tile cap)
AF = mybir.ActivationFunctionType
OP = mybir.AluOpType


# ---------------------------------------------------------------- prof hook
def _install_profhook():
    import types

    if "antenv.axon_hooks" in sys.modules:
        return
    try:
        from trn_agent_boot.trn_boot import _ntff_profile_via_ctypes
    except Exception:
        return
    mod = types.ModuleType("antenv.axon_hooks")
    mod._hook = None
    mod.set_axon_ntff_profile_hook = lambda h: setattr(mod, "_hook", h)
    mod.get_axon_ntff_profile_hook = lambda: mod._hook
    sys.modules["antenv.axon_hooks"] = mod
    try:
        mod._hook = _ntff_profile_via_ctypes("/opt/axon/libaxon_pjrt.so")
    except Exception:
        mod._hook = None


# ---------------------------------------------------------------- wait split
def _split_waits(nc, max_waits=1):
    n_added = 0
    for fn in nc.m.functions:
        for blk in fn.blocks:
            new_insts = []
            for inst in blk.instructions:
                si = getattr(inst, "sync_info", None)
                waits = list(si.on_wait) if si is not None and si.on_wait else []
                if len(waits) > max_waits:
                    extra = waits[:-max_waits]
                    for i in range(0, len(extra), max_waits):
                        chunk = extra[i : i + max_waits]
                        nop = mybir.InstNoOp(
                            name=f"{inst.name}_wsplit{n_added}",
                            engine=inst.engine,
                            ins=[],
                            outs=[],
                            sync_info=mybir.SyncInfo(on_wait=chunk, on_update=[]),
                        )
                        n_added += 1
                        new_insts.append(nop)
                    si.on_wait = waits[-max_waits:]
                new_insts.append(inst)
            blk.instructions = new_insts
    return n_added


# ---------------------------------------------------------------- host prep
def _prep(x, edge_index, batch, ncores):
    N = x.shape[0]
    own = N // ncores
    BLK = ncores * P
    ngroups = (N + BLK - 1) // BLK

    src = np.concatenate([edge_index[0].astype(np.int64), np.arange(N)])
    dst = np.concatenate([edge_index[1].astype(np.int64), np.arange(N)])
    deg = np.bincount(dst, minlength=N)

    order = np.argsort(-deg, kind="stable")
    rank_of = np.empty(N, np.int64)
    rank_of[order] = np.arange(N)
    g_of = rank_of // BLK
    within = rank_of % BLK
    c_of = within % ncores
    p_of = within // ncores
    row_of = c_of * own + g_of * P + p_of     # new global row id

    S_g = np.zeros(ngroups, np.int64)
    np.maximum.at(S_g, g_of, deg)
    S_g = np.maximum(S_g, 1)
    base_g = np.zeros(ngroups, np.int64)
    base_g[1:] = np.cumsum(S_g)[:-1]
    nchunk = int(S_g.sum())

    # per-edge slot: s = rank within dst's edge list
    o = np.argsort(dst, kind="stable")
    src_s, dst_s = src[o], dst[o]
    starts = np.zeros(N + 1, np.int64)
    starts[1:] = np.cumsum(np.bincount(dst_s, minlength=N))
    srank = np.arange(len(dst_s)) - starts[dst_s]

    ec = c_of[dst_s]
    ep = p_of[dst_s]
    ecol = base_g[g_of[dst_s]] + srank
    erow = row_of[src_s]

    srcidx = np.full((ncores, P, nchunk), N, np.int32)   # N = pad row
    srcidx[ec, ep, ecol] = erow

    # batch id per (core, group, p); 255 for pad partitions
    bl = np.full((ncores, P, ngroups), 255.0, np.float32)
    bl[c_of, p_of, g_of] = batch.astype(np.float32)
    batchloc = bl.astype(BF)

    # x rows in new order, transposed per core: xT[c] = x[rows of core c].T
    inv_rows = np.empty(N, np.int64)
    inv_rows[row_of] = np.arange(N)          # inv_rows[row] = old node id
    per_core = []
    for c in range(ncores):
        olds = inv_rows[c * own : (c + 1) * own]
        per_core.append(
            dict(
                xT_own=np.ascontiguousarray(x[olds].T).astype(BF),
                srcidx=np.ascontiguousarray(srcidx[c]),
                batchloc=np.ascontiguousarray(batchloc[c]),
            )
        )

    # vgroup split: interleaved column lists (round-robin across >=2 segs so
    # consecutive gathers hit alternating z tiles)
    vgroups = []
    for g in range(ngroups):
        W = int(S_g[g])
        nseg = max(2, -(-W // VCAP))
        segs = [list(range(j, W, nseg)) for j in range(nseg)]
        segs = [s for s in segs if s]
        vgroups.append(segs)

    meta = dict(N=N, own=own, ngroups=ngroups, S_g=S_g.tolist(),
                base_g=base_g.tolist(), nchunk=nchunk, vgroups=vgroups,
                ncores=ncores)
    return per_core, meta, row_of


def _fold_weights(W_l, b_l, W_r, b_r, att, bias, heads):
    """Device table weights in (c,h) layout with all folds. fperm maps device
    col -> reference col."""
    Ch = D // heads
    fperm = np.empty(D, np.int64)
    for c in range(Ch):
        for h in range(heads):
            fperm[c * heads + h] = h * Ch + c
    Wl_p, Wr_p = W_l[:, fperm], W_r[:, fperm]
    bl_p, br_p = b_l[fperm], b_r[fperm]
    bias_p = bias[fperm]
    att_f = att.reshape(heads, Ch)
    att_p = np.empty(D, np.float64)
    attblk = np.zeros((D, heads))
    for c in range(Ch):
        for h in range(heads):
            att_p[c * heads + h] = att_f[h, c]
            attblk[c * heads + h, h] = att_f[h, c]
    WA_l = 0.2 * (Wl_p @ attblk)
    WA_r = 0.2 * (Wr_p @ attblk)
    bA_l = 0.2 * (bl_p @ attblk)
    bA_r = 0.2 * (br_p @ attblk)
    if heads < EXT:
        WA_l = np.pad(WA_l, ((0, 0), (0, EXT - heads)))
        WA_r = np.pad(WA_r, ((0, 0), (0, EXT - heads)))
        bA_l = np.pad(bA_l, (0, EXT - heads))
        bA_r = np.pad(bA_r, (0, EXT - heads))
    Wl_ext = np.concatenate([Wl_p, WA_l], axis=1)
    Wr_ext = np.concatenate([Wr_p, WA_r], axis=1)
    brl = np.concatenate([bl_p + bias_p, bA_l])
    brr = np.concatenate([br_p - bias_p, bA_r])
    att08 = 0.8 * att_p
    return (Wl_ext.astype(np.float32), brl.astype(np.float32),
            Wr_ext.astype(np.float32), brr.astype(np.float32),
            att08.astype(np.float32), fperm)


# ---------------------------------------------------------------- kernel build
def _build(meta, debug=False):
    N = meta["N"]
    own = meta["own"]
    ngroups = meta["ngroups"]
    S_g = meta["S_g"]
    base_g = meta["base_g"]
    nchunk = meta["nchunk"]
    ncores = meta["ncores"]
    vgroups = meta["vgroups"]

    nc = bass.Bass(target_bir_lowering=False, debug=True, dynamic_dma_scratch_size=65536)

    # ---- inputs
    xT_in = nc.declare_dram_parameter("xT_own", [P, own], BF16, isOutput=False)
    srcidx_in = nc.declare_dram_parameter("srcidx", [P, nchunk], I32, isOutput=False)
    batchloc_in = nc.declare_dram_parameter("batchloc", [P, ngroups], BF16,
                                            isOutput=False)
    wnames = [
        ("W1l", [P, ROWW]), ("W1r", [P, ROWW]),
        ("W2l", [P, ROWW]), ("W2r", [P, ROWW]),
        ("brl1", [P, ROWW]), ("brr1", [P, ROWW]),
        ("brl2", [P, ROWW]), ("brr2", [P, ROWW]),
        ("att081", [P, D]), ("att082", [P, D]),
        ("iota64", [P, NUM_GRAPHS]),
        ("W3", [P, 10]), ("b3row", [1, 10]), ("ones1", [1, P]),
        ("onescol", [P, 1]), ("padrow", [1, ROWW]),
    ]
    w_in = {n: nc.declare_dram_parameter(n, sh, BF16, isOutput=False)
            for n, sh in wnames}
    out_t = nc.declare_dram_parameter("out", [NUM_GRAPHS, 10], F32, isOutput=True)

    # ---- internal DRAM
    xl1_own = nc.dram_tensor("xl1_own", [own, ROWW], BF16)
    xl1_full = nc.dram_tensor("xl1_full", [N + 1, ROWW], BF16, addr_space="Shared")
    xl2_own = nc.dram_tensor("xl2_own", [own, ROWW], BF16)
    xl2_full = nc.dram_tensor("xl2_full", [N + 1, ROWW], BF16, addr_space="Shared")
    pool_stage = nc.dram_tensor("pool_stage", [NUM_GRAPHS, D + 1], F32)
    pool_red = nc.dram_tensor("pool_red", [NUM_GRAPHS, D + 1], F32,
                              addr_space="Shared")

    # ---- persistent SBUF
    sb = {}

    def persist(name, shape, dtype):
        sb[name] = nc.alloc_sbuf_tensor(name, shape, dtype)
        return sb[name]

    xT_sb = persist("xT_sb", [P, own], BF16)
    srcidx_sb = persist("srcidx_sb", [P, nchunk], I32)
    batchloc_sb = persist("batchloc_sb", [P, ngroups], BF16)
    xr1_sb = persist("xr1_sb", [P, ngroups * ROWW], BF16)
    xr2_sb = persist("xr2_sb", [P, ngroups * ROWW], BF16)
    ident_sb = persist("ident_sb", [P, P], BF16)
    w_sb = {n: persist(n + "_sb", sh, BF16) for n, sh in wnames}

    def collective(kind, op, ins, outs):
        nc.gpsimd.collective_compute(
            kind, op, replica_groups=[list(range(ncores))], ins=ins, outs=outs
        )

    # ================= load constants =================
    with TileContext(nc) as tc:
        nc.sync.dma_start(out=xT_sb[:], in_=xT_in[:])
        nc.sync.dma_start(out=srcidx_sb[:], in_=srcidx_in[:])
        nc.sync.dma_start(out=batchloc_sb[:], in_=batchloc_in[:])
        for n, _sh in wnames:
            nc.sync.dma_start(out=w_sb[n][:], in_=w_in[n][:])
        with tc.tile_pool(name="idp", bufs=1) as idp:
            idt = idp.tile([P, P], F32)
            make_identity(nc, idt[:])
            nc.vector.tensor_copy(out=ident_sb[:], in_=idt[:])

    # ================= layer-1 tables =================
    def gsize(g):
        return P if g < ngroups - 1 or own % P == 0 else own % P

    with TileContext(nc) as tc:
        nc.sync.dma_start(out=xl1_full[N : N + 1, :], in_=w_sb["padrow"][:])
        nc.sync.dma_start(out=xl2_full[N : N + 1, :], in_=w_sb["padrow"][:])
        with (
            tc.tile_pool(name="t1s", bufs=3) as t1s,
            tc.tile_pool(name="t1p", bufs=4, space="PSUM") as t1p,
        ):
            for g in range(ngroups):
                w = gsize(g)
                psl = t1p.tile([P, ROWW], F32, tag="psl")
                psr = t1p.tile([P, ROWW], F32, tag="psr")
                nc.tensor.matmul(out=psl[:w, :], lhsT=xT_sb[:, g * P : g * P + w],
                                 rhs=w_sb["W1l"][:], start=True, stop=False)
                nc.tensor.matmul(out=psr[:w, :], lhsT=xT_sb[:, g * P : g * P + w],
                                 rhs=w_sb["W1r"][:], start=True, stop=False)
                nc.tensor.matmul(out=psl[:w, :], lhsT=ident_sb[:, :w],
                                 rhs=w_sb["brl1"][:], start=False, stop=True)
                nc.tensor.matmul(out=psr[:w, :], lhsT=ident_sb[:, :w],
                                 rhs=w_sb["brr1"][:], start=False, stop=True)
                st = t1s.tile([P, ROWW], BF16, tag="st")
                nc.scalar.activation(out=st[:w, :], in_=psl[:w, :], func=AF.Copy)
                nc.sync.dma_start(out=xl1_own[g * P : g * P + w, :], in_=st[:w, :])
                nc.vector.tensor_copy(
                    out=xr1_sb[:w, g * ROWW : (g + 1) * ROWW], in_=psr[:w, :])

    # ================= edge layer =================
    def edge_layer(tc, gv, work, wp, scp, pp, ep, aggp, tpsum, xl_full, xr_t,
                   att_t, heads, g, is_l1, pool_psum):
        """process one group g of one layer."""
        W = S_g[g]
        segs = vgroups[g]
        base = base_g[g]
        wsz = gsize(g)
        NH = heads
        C = D // heads

        agg = aggp.tile([P, D], F32, tag="agg")
        den = ep.tile([P, NH], F32, tag="den")
        # hoisted gathers, interleaved across the segments' z tiles
        zts = []
        for _si in range(len(segs)):
            zt = gv.tile([P, VCAP * ROWW], BF16, tag="z")
            zts.append(zt)
        maxV = max(len(s) for s in segs)
        for k in range(maxV):
            for si, seg in enumerate(segs):
                if k < len(seg):
                    nc.gpsimd.indirect_dma_start(
                        out=zts[si][:, k * ROWW : (k + 1) * ROWW], out_offset=None,
                        in_=xl_full[:],
                        in_offset=bass.IndirectOffsetOnAxis(
                            ap=srcidx_sb[:, base + seg[k] : base + seg[k] + 1],
                            axis=0),
                    )
        ztiles, ptiles = [], []
        for vi, seg in enumerate(segs):
            V = len(seg)
            z = zts[vi]
            zv = z[:].rearrange("p (s q) -> p s q", q=ROWW)[:, :V, :]
            # z += xr broadcast
            nc.vector.tensor_tensor(
                out=zv, in0=zv,
                in1=xr_t[:, g * ROWW : (g + 1) * ROWW]
                    .unsqueeze(1).broadcast_to([P, V, ROWW]),
                op=OP.add)
            # lr = relu(z feats) on ACT
            m = work.tile([P, VCAP * D], BF16, tag="m")
            mv3 = m[:].rearrange("p (s d) -> p s d", d=D)[:, :V, :]
            nc.scalar.activation(out=mv3, in_=zv[:, :, 0:D], func=AF.Relu)
            # m = lr * att08 broadcast
            nc.vector.tensor_tensor(
                out=mv3, in0=mv3,
                in1=att_t[:].unsqueeze(1).broadcast_to([P, V, D]),
                op=OP.mult)
            # tree reduce over c (innermost h for L1; flat halving for L2)
            if is_l1:
                mt = m[:].rearrange("p (s c h) -> p s c h", c=C, h=NH)[:, :V]
                half = C // 2
                while half >= 1:
                    nc.vector.tensor_tensor(
                        out=mt[:, :, 0:half, :], in0=mt[:, :, 0:half, :],
                        in1=mt[:, :, half : 2 * half, :], op=OP.add)
                    half //= 2
                tout = mt[:, :, 0, :]                       # [P, V, NH]
            else:
                mt = m[:].rearrange("p (s c) -> p s c", c=D)[:, :V]
                half = D // 2
                while half >= 8:
                    nc.vector.tensor_tensor(
                        out=mt[:, :, 0:half], in0=mt[:, :, 0:half],
                        in1=mt[:, :, half : 2 * half], op=OP.add)
                    half //= 2
                tout = None                                  # needs final reduce
            sc = scp.tile([P, VCAP * NH], BF16, tag="sc")
            scv = sc[:].rearrange("p (s h) -> p s h", h=NH)[:, :V]
            if is_l1:
                nc.vector.tensor_tensor(
                    out=scv, in0=tout, in1=zv[:, :, D : D + NH], op=OP.add)
            else:
                t8 = ep.tile([P, VCAP], F32, tag="t8")
                nc.vector.tensor_reduce(
                    out=t8[:, :V], in_=mt[:, :, 0:8],
                    axis=mybir.AxisListType.X, op=OP.add)
                nc.vector.tensor_tensor(
                    out=scv, in0=t8[:, :V].unsqueeze(2),
                    in1=zv[:, :, D : D + NH], op=OP.add)
            # p = exp(score)
            pt = pp.tile([P, VCAP * NH], F32 if not is_l1 else BF16, tag="pt")
            pv = pt[:].rearrange("p (s h) -> p s h", h=NH)[:, :V]
            nc.scalar.activation(out=pv, in_=scv, func=AF.Exp)
            # den partial
            if vi == 0:
                nc.vector.tensor_reduce(
                    out=den[:], in_=pt[:].rearrange("p (s h) -> p h s", h=NH)[:, :, :V],
                    axis=mybir.AxisListType.X, op=OP.add)
            else:
                dpart = ep.tile([P, NH], F32, tag="dpart")
                nc.vector.tensor_reduce(
                    out=dpart[:], in_=pt[:].rearrange("p (s h) -> p h s", h=NH)[:, :, :V],
                    axis=mybir.AxisListType.X, op=OP.add)
                nc.vector.tensor_tensor(out=den[:], in0=den[:], in1=dpart[:],
                                        op=OP.add)
            ztiles.append((z, seg))
            ptiles.append(pt)
        # reciprocal of den
        nc.vector.tensor_scalar(out=den[:], in0=den[:], scalar1=1e-30,
                                scalar2=None, op0=OP.max)
        rec = ep.tile([P, NH], F32, tag="rec")
        nc.vector.reciprocal(out=rec[:], in_=den[:])
        # phase B: alpha, w, agg
        emitted = 0
        for (z, seg), pt in zip(ztiles, ptiles):
            V = len(seg)
            zv = z[:].rearrange("p (s q) -> p s q", q=ROWW)[:, :V, :]
            wt = wp.tile([P, VCAP * D], BF16, tag="wt")
            if is_l1:
                al = scp.tile([P, VCAP * NH], BF16, tag="al")
                alv = al[:].rearrange("p (s h) -> p s h", h=NH)[:, :V]
                nc.vector.tensor_tensor(
                    out=alv, in0=pt[:].rearrange("p (s h) -> p s h", h=NH)[:, :V],
                    in1=rec[:].unsqueeze(1).broadcast_to([P, V, NH]), op=OP.mult)
                nc.vector.tensor_tensor(
                    out=wt[:].rearrange("p (s c h) -> p s c h", c=C, h=NH)[:, :V],
                    in0=zv[:, :, 0:D].rearrange("p s (c h) -> p s c h", h=NH),
                    in1=alv.unsqueeze(2).broadcast_to([P, V, C, NH]), op=OP.mult)
            else:
                al = pp.tile([P, VCAP], F32, tag="al2")
                nc.vector.tensor_scalar(out=al[:, :V], in0=pt[:, :V],
                                        scalar1=rec[:, 0:1], scalar2=None,
                                        op0=OP.mult)
                for k in range(V):
                    nc.vector.tensor_scalar(
                        out=wt[:, k * D : (k + 1) * D], in0=zv[:, k, 0:D],
                        scalar1=al[:, k : k + 1], scalar2=None, op0=OP.mult)
            for k in range(V):
                nc.tensor.matmul(out=agg[:], lhsT=ident_sb[:],
                                 rhs=wt[:, k * D : (k + 1) * D],
                                 start=(emitted == 0),
                                 stop=(emitted == W - 1))
                emitted += 1
        # epilogue: outn = agg - xr ; h = elu(outn)  (alpha already normalized)
        outn = ep.tile([P, D], F32, tag="outn")
        nc.vector.tensor_tensor(out=outn[:], in0=agg[:],
                                in1=xr_t[:, g * ROWW : g * ROWW + D], op=OP.subtract)
        neg = ep.tile([P, D], F32, tag="neg")
        nc.vector.tensor_scalar(out=neg[:], in0=outn[:], scalar1=0.0,
                                scalar2=None, op0=OP.min)
        en = ep.tile([P, D], F32, tag="en")
        nc.scalar.activation(out=en[:], in_=neg[:], func=AF.Exp)
        nc.vector.tensor_scalar(out=outn[:], in0=outn[:], scalar1=0.0,
                                scalar2=None, op0=OP.max)
        h = ep.tile([P, D], BF16, tag="h")
        nc.vector.tensor_tensor(out=h[:], in0=outn[:], in1=en[:], op=OP.add)
        nc.vector.tensor_scalar(out=h[:], in0=h[:], scalar1=-1.0, scalar2=None,
                                op0=OP.add)

        if is_l1:
            # transpose h -> h1T (lhsT for layer-2 builds)
            tps = tpsum.tile([P, P], BF16, tag="tps")
            nc.tensor.transpose(out=tps[:, :wsz], in_=h[:wsz, :],
                                identity=ident_sb[:wsz, :wsz])
            h1t = work.tile([P, P], BF16, tag="h1t")
            nc.scalar.activation(out=h1t[:, :wsz], in_=tps[:, :wsz], func=AF.Copy)
            psl = tpsum.tile([P, ROWW], F32, tag="psl2")
            psr = tpsum.tile([P, ROWW], F32, tag="psr2")
            nc.tensor.matmul(out=psl[:wsz, :], lhsT=h1t[:, :wsz],
                             rhs=w_sb["W2l"][:], start=True, stop=False)
            nc.tensor.matmul(out=psr[:wsz, :], lhsT=h1t[:, :wsz],
                             rhs=w_sb["W2r"][:], start=True, stop=False)
            nc.tensor.matmul(out=psl[:wsz, :], lhsT=ident_sb[:, :wsz],
                             rhs=w_sb["brl2"][:], start=False, stop=True)
            nc.tensor.matmul(out=psr[:wsz, :], lhsT=ident_sb[:, :wsz],
                             rhs=w_sb["brr2"][:], start=False, stop=True)
            st = work.tile([P, ROWW], BF16, tag="st2")
            nc.scalar.activation(out=st[:wsz, :], in_=psl[:wsz, :], func=AF.Copy)
            nc.sync.dma_start(out=xl2_own[g * P : g * P + wsz, :], in_=st[:wsz, :])
            nc.vector.tensor_copy(out=xr2_sb[:wsz, g * ROWW : (g + 1) * ROWW],
                                  in_=psr[:wsz, :])
        else:
            pind = work.tile([P, NUM_GRAPHS], BF16, tag="pind")
            nc.vector.tensor_tensor(
                out=pind[:],
                in0=batchloc_sb[:, g : g + 1].to_broadcast([P, NUM_GRAPHS]),
                in1=w_sb["iota64"][:], op=OP.is_equal)
            prhs = work.tile([P, D + 1], BF16, tag="prhs")
            nc.vector.tensor_copy(out=prhs[:wsz, 0:D], in_=h[:wsz, :])
            nc.gpsimd.memset(prhs[:, D : D + 1], 1.0)
            nc.tensor.matmul(out=pool_psum[:], lhsT=pind[:], rhs=prhs[:],
                             start=(g == 0), stop=(g == ngroups - 1))

    def run_layer(tc, xl_full, xr_t, att_name, heads, is_l1, pool_psum=None):
        with (
            tc.tile_pool(name="gv", bufs=4) as gv,
            tc.tile_pool(name="work", bufs=3) as work,
            tc.tile_pool(name="wp", bufs=2) as wp,
            tc.tile_pool(name="scp", bufs=3) as scp,
            tc.tile_pool(name="pp", bufs=4) as pp,
            tc.tile_pool(name="ep", bufs=2) as ep,
            tc.tile_pool(name="aggp", bufs=2, space="PSUM") as aggp,
            tc.tile_pool(name="tpsum", bufs=2, space="PSUM") as tpsum,
        ):
            for g in range(ngroups):
                edge_layer(tc, gv, work, wp, scp, pp, ep, aggp, tpsum,
                           xl_full, xr_t, w_sb[att_name], heads, g, is_l1,
                           pool_psum)

    with TileContext(nc) as tc:
        collective("AllGather", OP.bypass, [xl1_own[:]], [xl1_full[0:N, :]])
        run_layer(tc, xl1_full, xr1_sb, "att081", 8, True)

    with TileContext(nc) as tc:
        collective("AllGather", OP.bypass, [xl2_own[:]], [xl2_full[0:N, :]])
        with tc.tile_pool(name="poolp", bufs=1, space="PSUM") as poolp, \
             tc.tile_pool(name="pstg", bufs=1) as pstg:
            pool_psum = poolp.tile([NUM_GRAPHS, D + 1], F32)
            run_layer(tc, xl2_full, xr2_sb, "att082", 1, False,
                      pool_psum=pool_psum)
            stg = pstg.tile([NUM_GRAPHS, D + 1], F32)
            nc.scalar.activation(out=stg[:], in_=pool_psum[:], func=AF.Copy)
            nc.sync.dma_start(out=pool_stage[:], in_=stg[:])

    # ================= final head =================
    with TileContext(nc) as tc:
        collective("AllReduce", OP.add, [pool_stage[:]], [pool_red[:]])
        with (
            tc.tile_pool(name="fin", bufs=1) as fin,
            tc.tile_pool(name="finp", bufs=1, space="PSUM") as finp,
        ):
            red = fin.tile([NUM_GRAPHS, D + 1], F32)
            nc.sync.dma_start(out=red[:], in_=pool_red[:])
            cnt = fin.tile([NUM_GRAPHS, 1], F32)
            nc.vector.tensor_scalar(out=cnt[:], in0=red[:, D : D + 1],
                                    scalar1=1.0, scalar2=None, op0=OP.max)
            rc = fin.tile([NUM_GRAPHS, 1], F32)
            nc.vector.reciprocal(out=rc[:], in_=cnt[:])
            pooled = fin.tile([NUM_GRAPHS, D], BF16)
            nc.vector.tensor_tensor(out=pooled[:], in0=red[:, :D],
                                    in1=rc[:].to_broadcast([NUM_GRAPHS, D]),
                                    op=OP.mult)
            tp = finp.tile([P, NUM_GRAPHS], BF16)
            nc.tensor.transpose(out=tp[:], in_=pooled[:],
                                identity=ident_sb[:NUM_GRAPHS, :NUM_GRAPHS])
            pooledT = fin.tile([P, NUM_GRAPHS], BF16)
            nc.scalar.activation(out=pooledT[:], in_=tp[:], func=AF.Copy)
            ops = finp.tile([NUM_GRAPHS, 10], F32)
            nc.tensor.matmul(out=ops[:], lhsT=pooledT[:], rhs=w_sb["W3"][:],
                             start=True, stop=False)
            nc.tensor.matmul(out=ops[:], lhsT=w_sb["ones1"][:, :NUM_GRAPHS],
                             rhs=w_sb["b3row"][:], start=False, stop=True)
            fout = fin.tile([NUM_GRAPHS, 10], F32)
            nc.scalar.activation(out=fout[:], in_=ops[:], func=AF.Copy)
            nc.sync.dma_start(out=out_t[:], in_=fout[:])

    _split_waits(nc)
    return nc


# ---------------------------------------------------------------- entry point
def _run(x, edge_index, batch, W1_l, b1_l, W1_r, b1_r, att1, bias1,
         W2_l, b2_l, W2_r, b2_r, att2, bias2, W3, b3, ncores=NCORES,
         trace=False):
    x = np.asarray(x, np.float32)
    per_core, meta, row_of = _prep(x, np.asarray(edge_index),
                                   np.asarray(batch), ncores)

    W1e, brl1, W1re, brr1, att08_1, fperm1 = _fold_weights(
        np.asarray(W1_l, np.float32), np.asarray(b1_l, np.float32),
        np.asarray(W1_r, np.float32), np.asarray(b1_r, np.float32),
        np.asarray(att1, np.float32), np.asarray(bias1, np.float32), 8)
    W2_lp = np.asarray(W2_l, np.float32)[fperm1, :]
    W2_rp = np.asarray(W2_r, np.float32)[fperm1, :]
    W2e, brl2, W2re, brr2, att08_2, _f2 = _fold_weights(
        W2_lp, np.asarray(b2_l, np.float32), W2_rp,
        np.asarray(b2_r, np.float32), np.asarray(att2, np.float32),
        np.asarray(bias2, np.float32), 1)

    def rep(v):
        return np.tile(np.asarray(v, np.float32).reshape(1, -1), (P, 1)).astype(BF)

    consts = dict(
        W1l=W1e.astype(BF), W1r=W1re.astype(BF),
        W2l=W2e.astype(BF), W2r=W2re.astype(BF),
        brl1=rep(brl1), brr1=rep(brr1), brl2=rep(brl2), brr2=rep(brr2),
        att081=rep(att08_1), att082=rep(att08_2),
        iota64=np.tile(np.arange(NUM_GRAPHS, dtype=np.float32).reshape(1, -1),
                       (P, 1)).astype(BF),
        W3=np.asarray(W3, np.float32).astype(BF),
        b3row=np.asarray(b3, np.float32).reshape(1, 10).astype(BF),
        ones1=np.ones((1, P), np.float32).astype(BF),
        onescol=np.ones((P, 1), np.float32).astype(BF),
        padrow=np.full((1, ROWW), -1e4, np.float32).astype(BF),
    )
    nc = _build(meta)
    in_maps = []
    for c in range(ncores):
        m = dict(per_core[c])
        m.update(consts)
        in_maps.append(m)
    if trace:
        _install_profhook()
    res = run_bass_kernel_spmd(nc, in_maps, core_ids=list(range(ncores)),
                               trace=trace)
    return res.results[0]["out"].astype(np.float32), (res, per_core, meta, row_of)


def kernel(**inputs):
    out, _res = _run(**inputs)
    return out
